# revision 22
# baseline (speedup 1.0000x reference)
"""Trainium2 Bass kernel for nn_ConvProjector (conv3x3 -> ReLU -> conv3x3 -> ReLU
-> adaptive-avg-pool upsample 32x32 -> 687x1024 -> 1x1 conv 256->24 + bias).

Strategy:
  * The adaptive pool (linear) and the 1x1 conv (linear) commute: apply the
    256->24 channel reduction at 32x32 resolution first, then upsample only
    24 channels. The pooled tensor never materializes at 256 channels.
  * W axis: 1024 = 32*32 exactly -> every window has length 1 (pure
    replication). Done with a matmul against a 0/1 expansion matrix E.
  * H axis: 687 from 32 -> runs of 21/22 rows per input row; the last row of
    each run (except the final one) is the mean of two adjacent input rows.
    All replicated rows are written by ONE stride-0-source DMA (rw laid out
    c-major so the HBM offset is uniform in the partition index); averaged
    rows come from a second accumulating expansion matmul scaled by 0.5.
  * All matmuls run in fp16 (inputs pre-rounded on host; PSUM stays fp32);
    the device output buffer is fp16 and is upcast on the host.
  * Sharding: 8 cores, core k owns input rows 4k..4k+3 (+1 halo row) and
    produces its ~86 output rows. No collectives. Convs computed on a
    9-row x-slice per core (3x3 halos), channel-complete.
Output is assembled on the host from the per-core (24, 88, 1024) buffers.
"""
import sys

if '/opt/trn_rl_repo' not in sys.path:
    sys.path.insert(0, '/opt/trn_rl_repo')

import numpy as np

IN_C, MID_C, OUT_C = 576, 256, 24
H = W = 32
OUT_H, OUT_W = 687, 1024
NCORES = 8
P = 128
KC1 = 5           # ceil(576/128) input-channel chunks for conv1 (padded to 640)
KC2 = 2           # 256/128 chunks for conv2 / 1x1
MC = 2            # 256/128 output-channel chunks for conv1/conv2
W36 = 36          # padded row width (2 zero cols each side)
RX, R1, R2 = 9, 7, 5          # x rows / h1 rows / h2 (=r) rows per core
XBLK = RX * W36               # 324  per-kc x block
XSLACK = 16                   # rhs overrun slack so N can pad to 256
N1 = 256                      # conv1 matmul N (padded up from 248)
H1BLK = R1 * W36              # 252  per-mc h1 block
H1SLACK = 80
N2 = 256                      # conv2 matmul N (padded; valid span is 176)
NV2 = 176                     # valid h2 flat span per mc
RUN = 22                      # output rows per owned input row in core buffer
NBUF = 4 * RUN                # 88 buffer rows per core

_prog_cache = {}


def _h_runs():
    i = np.arange(OUT_H)
    s = (i * H) // OUT_H
    t = np.searchsorted(s, np.arange(H + 1), side='left')
    return s, t


def _build_program():
    import concourse.bass as bass
    import concourse.bacc as bacc
    import concourse.mybir as mybir
    from concourse.tile import TileContext

    f32 = mybir.dt.float32
    f16 = mybir.dt.float16
    nc = bacc.Bacc("TRN2", target_bir_lowering=False, debug=False,
                   num_devices=NCORES)

    xs_d = nc.dram_tensor("xs", [P, KC1 * XBLK + XSLACK], f16, kind="ExternalInput")
    w1_d = nc.dram_tensor("w1p", [P, 9 * KC1 * MC * P], f16, kind="ExternalInput")
    w2_d = nc.dram_tensor("w2p", [P, 9 * KC2 * MC * P], f16, kind="ExternalInput")
    wr_d = nc.dram_tensor("wrp", [P, KC2 * OUT_C], f16, kind="ExternalInput")
    b1_d = nc.dram_tensor("b1p", [P, MC], f32, kind="ExternalInput")
    b2_d = nc.dram_tensor("b2p", [P, MC], f32, kind="ExternalInput")
    br_d = nc.dram_tensor("brp", [96, 1], f32, kind="ExternalInput")
    em_d = nc.dram_tensor("emp", [32, OUT_W], f16, kind="ExternalInput")
    mk_d = nc.dram_tensor("mkp", [P, H1BLK], f16, kind="ExternalInput")
    out_d = nc.dram_tensor("outb", [OUT_C, NBUF, OUT_W], f16, kind="ExternalOutput")

    Relu = mybir.ActivationFunctionType.Relu
    Ident = mybir.ActivationFunctionType.Identity

    with TileContext(nc) as tc:
        with (
            tc.tile_pool(name="sb", bufs=1) as sb,
            tc.tile_pool(name="ps", bufs=1, space="PSUM") as psp,
        ):
            x_t = sb.tile([P, KC1 * XBLK + XSLACK], f16)
            # one tile per conv1 tap so matmuls start as soon as that tap's
            # weights land
            w1_ts = [sb.tile([P, KC1 * MC * P], f16, tag=f"w1_{t}",
                             name=f"w1t{t}") for t in range(9)]
            w2_ts = [sb.tile([P, 3 * KC2 * MC * P], f16, tag=f"w2_{t}",
                             name=f"w2t{t}") for t in range(3)]
            wr_t = sb.tile([P, KC2 * OUT_C], f16)
            b1_t = sb.tile([P, MC], f32)
            b2_t = sb.tile([P, MC], f32)
            br_t = sb.tile([96, 1], f32)
            em_t = sb.tile([32, OUT_W], f16)
            mk_t = sb.tile([P, H1BLK], f16)
            h1_t = sb.tile([P, MC * H1BLK + H1SLACK], f16)
            h2_t = sb.tile([P, MC * NV2], f16)
            rt_t = sb.tile([32, 2 * 96], f16)
            rw_t = sb.tile([96, OUT_W], f16)
            av_t = sb.tile([96, OUT_W], f16)

            nc.sync.dma_start(x_t[:], xs_d.ap())
            w1blk = KC1 * MC * P
            for t in range(9):
                nc.sync.dma_start(
                    w1_ts[t][:],
                    bass.AP(w1_d, t * w1blk,
                            [[9 * w1blk, P], [1, w1blk]]))
            nc.scalar.dma_start(b1_t[:], b1_d.ap())
            nc.scalar.dma_start(b2_t[:], b2_d.ap())
            nc.scalar.dma_start(br_t[:], br_d.ap())
            nc.scalar.dma_start(wr_t[:], wr_d.ap())
            nc.scalar.dma_start(em_t[:], em_d.ap())
            nc.scalar.dma_start(mk_t[:], mk_d.ap())
            w2blk = 3 * KC2 * MC * P
            for t in range(3):
                nc.scalar.dma_start(
                    w2_ts[t][:],
                    bass.AP(w2_d, t * w2blk,
                            [[3 * w2blk, P], [1, w2blk]]))

            # h1 pads must be zero; activation only writes valid 32-col spans.
            nc.vector.memset(h1_t[:], 0.0)


            # ---- conv1: 576 -> 256 over 7 rows --------------------------
            for mc in range(MC):
                ps1 = psp.tile([P, N1], f32, tag="cv")
                n_acc = 9 * KC1
                i_acc = 0
                for tap in range(9):
                    ky, kx = tap // 3, tap % 3
                    off = ky * W36 + kx + 1
                    for kc in range(KC1):
                        nc.tensor.matmul(
                            ps1[:, :],
                            lhsT=w1_ts[tap][:, (kc * MC + mc) * P:
                                            (kc * MC + mc) * P + P],
                            rhs=x_t[:, kc * XBLK + off: kc * XBLK + off + N1],
                            start=(i_acc == 0), stop=(i_acc == n_acc - 1),
                        )
                        i_acc += 1
                # ReLU(x + b) into the valid 32-wide spans of padded h1 rows
                src = bass.AP(ps1.tensor, ps1.offset,
                              [[N1, P], [W36, R1], [1, 32]])
                dstb = h1_t[:, :]
                dst = bass.AP(dstb.tensor, dstb.offset + mc * H1BLK + 2,
                              [[MC * H1BLK + H1SLACK, P], [W36, R1], [1, 32]])
                nc.scalar.activation(dst, src, Relu, bias=b1_t[:, mc:mc + 1])

            # zero h1 rows that lie outside the global image (cores 0 and 7)
            for mc in range(MC):
                nc.vector.tensor_mul(h1_t[:, mc * H1BLK:(mc + 1) * H1BLK],
                                     h1_t[:, mc * H1BLK:(mc + 1) * H1BLK],
                                     mk_t[:, :])

            # ---- conv2: 256 -> 256 over 5 rows --------------------------
            for mc in range(MC):
                ps2 = psp.tile([P, N2], f32, tag="cv")
                n_acc = 9 * KC2
                i_acc = 0
                for tap in range(9):
                    ky, kx = tap // 3, tap % 3
                    off = ky * W36 + kx + 1
                    for kc in range(KC2):
                        nc.tensor.matmul(
                            ps2[:, :],
                            lhsT=w2_ts[tap // 3][:, ((tap % 3) * KC2 + kc) * MC * P
                                                 + mc * P:
                                                 ((tap % 3) * KC2 + kc) * MC * P
                                                 + mc * P + P],
                            rhs=h1_t[:, kc * H1BLK + off: kc * H1BLK + off + N2],
                            start=(i_acc == 0), stop=(i_acc == n_acc - 1),
                        )
                        i_acc += 1
                src2 = bass.AP(ps2.tensor, ps2.offset,
                               [[N2, P], [W36, R2], [1, 32]])
                h2b = h2_t[:, :]
                dst2 = bass.AP(h2b.tensor, h2b.offset + mc * NV2,
                               [[MC * NV2, P], [W36, R2], [1, 32]])
                nc.scalar.activation(dst2, src2, Relu, bias=b2_t[:, mc:mc + 1])

            # ---- 1x1 conv 256 -> 24, transposed into (w, (h, c)) --------
            psr = psp.tile([32, R2 * OUT_C], f32, tag="psr")
            for h in range(R2):
                for kc in range(KC2):
                    nc.tensor.matmul(
                        psr[:, h * OUT_C:(h + 1) * OUT_C],
                        lhsT=h2_t[:, kc * NV2 + h * W36: kc * NV2 + h * W36 + 32],
                        rhs=wr_t[:, kc * OUT_C:(kc + 1) * OUT_C],
                        start=(kc == 0), stop=(kc == KC2 - 1),
                    )
            # reshuffle (h, c) -> c-major 4c+h twice (rows h and h+1) so the
            # expansion lhsT slices are contiguous and downstream DMA offsets
            # are uniform in the partition index
            psrb = psr[:, :]
            rtb = rt_t[:, :]
            for half in range(2):
                src = bass.AP(psrb.tensor, psrb.offset + half * OUT_C,
                              [[R2 * OUT_C, 32], [OUT_C, 4], [1, OUT_C]])
                dst = bass.AP(rtb.tensor, rtb.offset + half * 96,
                              [[2 * 96, 32], [1, 4], [4, OUT_C]])
                nc.vector.tensor_copy(dst, src)

            # ---- W expansion 32 -> 1024 (+ averaged-row variant) --------
            # lhsT free layout (c:24 stride 5, h:4 stride 1) -> M = 96
            psw = psp.tile([96, OUT_W], f32, tag="psw")
            psa = psp.tile([96, OUT_W], f32, tag="psa")
            lhs_pure = rt_t[:, 0:96]
            lhs_next = rt_t[:, 96:192]
            for j in range(2):
                nc.tensor.matmul(psw[:, j * 512:(j + 1) * 512],
                                 lhsT=lhs_pure,
                                 rhs=em_t[:, j * 512:(j + 1) * 512],
                                 start=True, stop=True)
                nc.tensor.matmul(psa[:, j * 512:(j + 1) * 512],
                                 lhsT=lhs_pure,
                                 rhs=em_t[:, j * 512:(j + 1) * 512],
                                 start=True, stop=False)
                nc.tensor.matmul(psa[:, j * 512:(j + 1) * 512],
                                 lhsT=lhs_next,
                                 rhs=em_t[:, j * 512:(j + 1) * 512],
                                 start=False, stop=True)
            # ---- H expansion ------------------------------------------
            # partition p = 4c + h; HBM offset of row (c, 22h) = 22528 * p,
            # so ONE stride-0-source DMA writes all 96x21 replicated rows.
            nc.scalar.activation(rw_t[:, :], psw[:, :], Ident,
                                 bias=br_t[:, 0:1])
            nc.scalar.activation(av_t[:, :], psa[:, :], Ident, scale=0.5,
                                 bias=br_t[:, 0:1])
            rwb = rw_t[:, :]
            src = bass.AP(rwb.tensor, rwb.offset,
                          [[OUT_W, 96], [0, RUN - 1], [1, OUT_W]])
            dst = bass.AP(out_d, 0,
                          [[RUN * OUT_W, 96], [OUT_W, RUN - 1], [1, OUT_W]])
            nc.sync.dma_start(dst, src)
            avb = av_t[:, :]
            srca = bass.AP(avb.tensor, avb.offset, [[OUT_W, 96], [1, OUT_W]])
            dsta = bass.AP(out_d, (RUN - 1) * OUT_W,
                           [[RUN * OUT_W, 96], [1, OUT_W]])
            nc.scalar.dma_start(dsta, srca)

    nc.compile()
    return nc


def _pack_inputs(x, w1, b1, w2, b2, wr, br):
    x = np.asarray(x, np.float32)
    w1 = np.asarray(w1, np.float32)
    w2 = np.asarray(w2, np.float32)
    wr = np.asarray(wr, np.float32)
    b1 = np.asarray(b1, np.float32)
    b2 = np.asarray(b2, np.float32)
    br = np.asarray(br, np.float32)

    xp = np.zeros((NCORES, P, KC1, RX, W36), np.float16)
    xv = x[0]  # (576, 32, 32)
    for k in range(NCORES):
        for r in range(RX):
            g = 4 * k - 2 + r
            if 0 <= g < H:
                blkv = xv[:, g, :]  # (576, 32)
                xp[k, :, :4, r, 2:34] = blkv[:512].reshape(4, P, W).transpose(1, 0, 2)
                xp[k, :64, 4, r, 2:34] = blkv[512:]
    xp = xp.reshape(NCORES, P, KC1 * XBLK)
    xp = np.concatenate([xp, np.zeros((NCORES, P, XSLACK), np.float16)], axis=2)

    # w1: [p, tap, kc, mc, m] = w1[mc*128+m, kc*128+p, ky, kx]
    w1p = np.zeros((P, 9, KC1, MC, P), np.float16)
    w1v = w1.transpose(2, 3, 1, 0).reshape(9, IN_C, MID_C)  # (tap, ci, co)
    w1p[:, :, :4, :, :] = (
        w1v[:, :512, :].reshape(9, 4, P, MC, P).transpose(2, 0, 1, 3, 4))
    w1p[:64, :, 4, :, :] = w1v[:, 512:, :].reshape(9, 64, MC, P).transpose(1, 0, 2, 3)
    w1p = w1p.reshape(P, 9 * KC1 * MC * P)

    w2p = np.zeros((P, 9, KC2, MC, P), np.float16)
    w2v = w2.transpose(2, 3, 1, 0).reshape(9, MID_C, MID_C)
    w2p[:, :, :, :, :] = (
        w2v.reshape(9, KC2, P, MC, P).transpose(2, 0, 1, 3, 4))
    w2p = w2p.reshape(P, 9 * KC2 * MC * P)

    wrp = wr.T.reshape(KC2, P, OUT_C).transpose(1, 0, 2).reshape(P, KC2 * OUT_C)
    wrp = np.ascontiguousarray(wrp, np.float16)
    b1p = b1.reshape(MC, P).T.copy()
    b2p = b2.reshape(MC, P).T.copy()
    # bias for partition p = 4c + h -> br[p // 4]
    brp = np.repeat(br, 4).reshape(96, 1).copy()
    em = (np.arange(OUT_W) // 32 == np.arange(32)[:, None]).astype(np.float16)

    mkp = np.zeros((NCORES, P, R1, W36), np.float16)
    for k in range(NCORES):
        for r in range(R1):
            if 0 <= 4 * k - 1 + r < H:
                mkp[k, :, r, :] = 1.0
    mkp = mkp.reshape(NCORES, P, H1BLK)

    shared = dict(w1p=w1p, w2p=w2p, wrp=wrp, b1p=b1p, b2p=b2p, brp=brp,
                  emp=em)
    in_maps = []
    for k in range(NCORES):
        m = dict(shared)
        m["xs"] = np.ascontiguousarray(xp[k])
        m["mkp"] = np.ascontiguousarray(mkp[k])
        in_maps.append(m)
    return in_maps


def kernel(x, w1, b1, w2, b2, wr, br):
    from concourse.bass_utils import run_bass_kernel_spmd

    if "nc" not in _prog_cache:
        _prog_cache["nc"] = _build_program()
    nc = _prog_cache["nc"]

    in_maps = _pack_inputs(x, w1, b1, w2, b2, wr, br)
    res = run_bass_kernel_spmd(nc, in_maps, list(range(NCORES)))

    _, t = _h_runs()
    out = np.empty((1, OUT_C, OUT_H, OUT_W), np.float32)
    for k in range(NCORES):
        buf = res.results[k]["outb"].astype(np.float32)  # (24, 88, 1024)
        for hl in range(4):
            h = 4 * k + hl
            n = t[h + 1] - t[h]
            if h < H - 1:
                out[0, :, t[h]:t[h] + n - 1, :] = buf[:, RUN * hl:RUN * hl + n - 1, :]
                out[0, :, t[h] + n - 1, :] = buf[:, RUN * hl + RUN - 1, :]
            else:
                out[0, :, t[h]:t[h] + n, :] = buf[:, RUN * hl:RUN * hl + n, :]
    return out
